# revision 21
# baseline (speedup 1.0000x reference)
"""Trainium2 Bass kernel for nn_NeuralSurface (8-layer MLP SDF with harmonic
embedding + skip concat), data-parallel over 8 NeuronCores.

Layout strategy: activations kept transposed in SBUF ([features, points]),
weights stationary fp16, PE matmuls K/M-chunked to 128. ReLU+bias split
between ScalarE (activation Relu w/ bias) and VectorE (tensor_scalar
add+max) reading PSUM. n-tiles processed in pairs so the PE always has
independent matmul work while ReLUs complete.

v6.5:
- PE array tiling reclaims the padded matmuls. The pair's two 512-point
  halves are split across PE row-tiles: half A's 39-row embedding lives in
  SBUF partitions 0:39, half B's in 64:103, with w0/w4e duplicated
  host-side in both partition ranges. L0 and L4's emb-chunk run as K=39
  (64,128)-tiled matmuls, two concurrent (T0/T8), halving their wall time
  (trace: dual-MM starts 4ns apart, block 604ns vs 1516 serial).
- SDF matmuls (M=1) run (128,32) col-tiled at column positions 0 and 32
  of ONE psum bank as two accumulation groups (the has_written clear of
  start=True is write-region-scoped, verified empirically); the two tiles
  stream concurrently.
- The harmonic embedding sin/cos is computed ON HOST and DMA'd as a
  [39, NPC] fp16 tensor -- same HBM bytes as shipping phases, no on-chip
  Sin at all (removes 1.44us/pair of ScalarE work and the ACT-LUT-load
  dependency from the startup critical path).
- The SDF drain is a single 33-partition ScalarE op at the li==1 slot
  (strided partition APs are illegal on engines but legal on the output
  DMA, which picks rows {0,32}); placed to dodge the L1-L2 FIFO
  congestion. VectorE carries relus only.
- Within each layer-half, both ka matmuls are emitted before the kb pair
  (ci-outer loop) so every kb issue point lands after its DVE relu
  completes -- this removed the last per-pair boundary stall (the DVE
  L7A1/L7B1/L0A1 serial chain).
- Warmup matmuls feed on a gpsimd-memset tile (no DMA dependency).
Disproven on HW (do not retry): fp8 anywhere (one e4m3 layer alone
breaks the 2e-2 gate); merging L4e into the boundary R-region with a
dedicated 4-bank hold pool (+16us: the 4-bank flow rotation serializes
even though same-event-synchronized in theory; the 8-bank shared ring
is load-bearing).
"""

import numpy as np

import concourse.bacc as bacc
import concourse.mybir as mybir
import concourse.tile as tile
from concourse.bass_utils import run_bass_kernel_spmd

AF = mybir.ActivationFunctionType
ALU = mybir.AluOpType
F32 = mybir.dt.float32
F16 = mybir.dt.float16

N_CORES = 8
N = 262144
NPC = N // N_CORES  # 32768 points per core
NT = 512  # points per n-tile (PSUM bank / fp32 moving-operand limit)
PAIRS = NPC // (2 * NT)  # 32
H = 256
E = 39
NHARM = 6
TWO_PI = float(2.0 * np.pi)

# packed weight tensor column offsets ([128, WCOLS] fp16; K on partitions)
# w0 / w4e are dual-row packed: rows 0:39 AND rows 64:103 hold the same
# [39, 256] weight so row-tiles T0 (partitions 0:64) and T8 (64:128) can
# each load their half's stationary.
OFF_W0 = 0
_K_LAYERS = (1, 2, 3, 5, 6, 7)
OFF_WK = {li: 256 + j * 512 for j, li in enumerate(_K_LAYERS)}  # ka, kb halves
OFF_W4E = 256 + 6 * 512  # 3328
OFF_W4A = OFF_W4E + 256
OFF_W4B = OFF_W4A + 256
OFF_SDF = OFF_W4B + 256  # 2 cols: col 0 = wsdf[0:128], col 1 = wsdf[128:256]
WCOLS = OFF_SDF + 2  # 4354

# ReLU engine split: half 0 -> ACT, half 1 -> DVE (even split; each PSUM
# pair drains through two engines in parallel).
DVE_RELU = {(li, 1): True for li in range(8)}

_CACHED = {}


def _build():
    nc = bacc.Bacc("TRN2")

    embf = nc.dram_tensor("embf", [E, NPC], F16, kind="ExternalInput").ap()
    wpack = nc.dram_tensor("wpack", [128, WCOLS], F16, kind="ExternalInput").ap()
    bmat = nc.dram_tensor("bmat", [128, 16], F32, kind="ExternalInput").ap()
    bsdf1 = nc.dram_tensor("bsdf1", [128, 1], F32, kind="ExternalInput").ap()
    # 2-D output (1-D ExternalOutput tensors fail NEFF load under bass2jax)
    out_o = nc.dram_tensor("out_o", [NPC // NT, NT], F32, kind="ExternalOutput").ap()

    with tile.TileContext(nc) as tc:
        with (
            tc.tile_pool(name="wp", bufs=1) as wp,
            tc.tile_pool(name="ep", bufs=3) as ep,
            tc.tile_pool(name="hp", bufs=4) as hp,
            tc.tile_pool(name="op", bufs=4) as op_,
            tc.tile_pool(name="pp", bufs=8, space="PSUM") as pp,
        ):
            # ---- one-time weight / const loads ----
            # warmup feedstock: memset (no DMA dependency) so the PE p-state
            # ramp starts as soon as the queues boot.
            wdum = wp.tile([128, 512], F16, name="wdum")
            nc.gpsimd.memset(wdum, 0.0)
            # packed weights in 3 chunks on the scalar queue (idle at boot),
            # so the sync queue's first issue is pair-0's embedding DMA (the
            # L0 critical path).
            wps = wp.tile([128, WCOLS], F16, name="wps")
            nc.scalar.dma_start(out=wps[:, 0:1024], in_=wpack[:, 0:1024])
            # bias/bsdf constants ride the gpsimd queue
            bms = wp.tile([128, 16], F32, name="bms")
            nc.gpsimd.dma_start(out=bms, in_=bmat)
            bsdfs = wp.tile([128, 1], F32, name="bsdfs")
            nc.gpsimd.dma_start(out=bsdfs, in_=bsdf1)

            w0s = wps[:, OFF_W0:OFF_W0 + 256]  # dual-row [0:39]/[64:103]
            wks = {
                li: (
                    wps[:, OFF_WK[li]:OFF_WK[li] + 256],
                    wps[:, OFF_WK[li] + 256:OFF_WK[li] + 512],
                )
                for li in _K_LAYERS
            }
            w4es = wps[:, OFF_W4E:OFF_W4E + 256]  # dual-row
            w4as = wps[:, OFF_W4A:OFF_W4A + 256]
            w4bs = wps[:, OFF_W4B:OFF_W4B + 256]
            wsdf_a = wps[:, OFF_SDF:OFF_SDF + 1]  # [128, 1]
            wsdf_b = wps[:, OFF_SDF + 1:OFF_SDF + 2]

            # ---- PE p-state warmup ----
            # the first real matmul waits for the pair-0 embedding DMA;
            # without these the first ~11 real matmuls run at 1.2GHz (HAM
            # un-gates the PE clock only after ~3.4us of sustained busy).
            pwt = pp.tile([128, NT], F32, tag="mm", name="pwt")
            for _ in range(7):
                nc.tensor.matmul(pwt, wdum[:, 0:128], wdum, start=True, stop=True)

            # previous pair's state for the deferred SDF emission
            h7_prev = None
            psf_prev = None

            def emit_sdf(h7):
                # (128,32) col-tiled, ONE psum bank, TWO accumulation
                # groups at disjoint partition rows: the has_written clear
                # of start=True is write-region-scoped (a start=False MM on
                # an untouched row was observed accumulating onto the
                # bank's stale bits), so each half carries its own
                # start=True on its own row. Half A accumulates at
                # partition 0 (tile (0,0)), half B at partition 32 (tile
                # (0,32)); the two tiles stream concurrently.
                sp = pp.tile([128, NT], F32, tag="mm", name="spsdf")
                nc.tensor.matmul(
                    sp[0:1, :], wsdf_a, h7[:, bass_ts(0, NT)], start=True, stop=False
                )
                nc.tensor.matmul(
                    sp[32:33, :], wsdf_a, h7[:, bass_ts(2, NT)], start=True, stop=False
                )
                nc.tensor.matmul(
                    sp[0:1, :], wsdf_b, h7[:, bass_ts(1, NT)], start=False, stop=True
                )
                nc.tensor.matmul(
                    sp[32:33, :], wsdf_b, h7[:, bass_ts(3, NT)], start=False, stop=True
                )
                return sp

            def emit_sdf_out(pq, sp):
                # single partition-preserving drain: strided partition AP
                # reads rows {0,32} (the A/B dot-product rows), adds bsdf,
                # writes the same partitions of an SBUF tile; one 2-row DMA.
                # On DVE: ACT showed a ~487ns/pair stall when both drains
                # sat in its FIFO during the congested L1-L2 stretch.
                # Engine ops forbid strided partition APs (BIR verifier),
                # but engine cost is free-size-driven: process all 33
                # partitions (rows 1..31 are stale-psum junk, never read
                # downstream) and let the DMA pick rows {0,32} with a
                # strided AP. On ScalarE: it has the most slack (DVE
                # placement delayed L1's relus and stalled L2's kb MMs).
                ot = op_.tile([33, NT], F32, tag="ot")
                nc.scalar.activation(
                    ot, sp[0:33, :], AF.Identity, bias=bsdfs[0:33, 0:1]
                )
                nc.sync.dma_start(
                    out=out_o[2 * pq:2 * pq + 2, :], in_=ot[0:33:32, :]
                )

            def emit_embedding(p):
                s = p * 2 * NT
                # embd holds half A's 39-row host-computed embedding
                # ([sin|cos|xyz], fp16) in partitions 0:39 and half B's in
                # 64:103 so the K=39 matmuls run row-tiled on T0/T8 with no
                # zero padding and no on-chip trig.
                embd = ep.tile([128, NT], F16, tag="embd", name="embd")
                nc.sync.dma_start(out=embd[0:39, :], in_=embf[:, s:s + NT])
                nc.sync.dma_start(out=embd[64:103, :], in_=embf[:, s + NT:s + 2 * NT])
                return embd

            embd = emit_embedding(0)
            # remaining weight chunks: issued behind pair-0's embedding DMAs
            # on the scalar queue; wk2b..wk6a land before L2 of pair 0 needs
            # them
            nc.scalar.dma_start(out=wps[:, 1024:2560], in_=wpack[:, 1024:2560])
            nc.scalar.dma_start(out=wps[:, 2560:WCOLS], in_=wpack[:, 2560:WCOLS])
            emb_next = None
            for p in range(PAIRS):
                # ---- MLP layers ----
                # h tile layout: [128, 4*NT]: A-half0, A-half1, B-half0, B-half1
                h3 = None
                h_prev = None
                for li in range(8):
                    h = hp.tile([128, 4 * NT], F16, tag="h")
                    ps = {(hx, m): pp.tile([128, NT], F32, tag="mm", name="psmm")
                          for hx in range(2) for m in range(2)}
                    if li == 0 or li == 4:
                        # K=39 emb chunks, (64,128) row-tiled: T0 computes
                        # half A (emb rows 0:39), T8 half B (rows 64:103);
                        # T0/T8 alternated so both tiles stay fed.
                        wt = w0s if li == 0 else w4es
                        stop = li == 0  # L4 accumulates main chunks after
                        for m in range(2):
                            nc.tensor.matmul(
                                ps[(0, m)], wt[0:39, bass_ts(m, 128)],
                                embd[0:39, :], start=True, stop=stop,
                            )
                            nc.tensor.matmul(
                                ps[(1, m)], wt[64:103, bass_ts(m, 128)],
                                embd[64:103, :], start=True, stop=stop,
                            )
                        if li == 4:
                            for hx in range(2):
                                for m in range(2):
                                    nc.tensor.matmul(
                                        ps[(hx, m)], w4as[:, bass_ts(m, 128)],
                                        h3[:, bass_ts(2 * hx, NT)],
                                        start=False, stop=False,
                                    )
                                for m in range(2):
                                    nc.tensor.matmul(
                                        ps[(hx, m)], w4bs[:, bass_ts(m, 128)],
                                        h3[:, bass_ts(2 * hx + 1, NT)],
                                        start=False, stop=True,
                                    )
                    else:
                        chunks = [
                            (wks[li][0], lambda hx, hp_=h_prev: hp_[:, bass_ts(2 * hx, NT)]),
                            (wks[li][1], lambda hx, hp_=h_prev: hp_[:, bass_ts(2 * hx + 1, NT)]),
                        ]
                        last = len(chunks) - 1
                        # ci-outer: both ka matmuls (ACT-relu inputs) issue
                        # before the kb pair (DVE-relu inputs), pushing each
                        # kb issue point past its DVE relu's completion --
                        # the pair-boundary DVE chain was the binding stall.
                        for hx in range(2):
                            for ci, (wt, rhs) in enumerate(chunks):
                                for m in range(2):
                                    nc.tensor.matmul(
                                        ps[(hx, m)], wt[:, bass_ts(m, 128)], rhs(hx),
                                        start=(ci == 0), stop=(ci == last),
                                    )
                    # ReLU + bias -> h
                    for half_x in range(2):
                        for m in range(2):
                            dst = h[:, bass_ts(2 * half_x + m, NT)]
                            bias_ap = bms[:, li * 2 + m:li * 2 + m + 1]
                            if DVE_RELU.get((li, m), False):
                                nc.vector.tensor_scalar(
                                    dst, ps[(half_x, m)], bias_ap, 0.0,
                                    op0=ALU.add, op1=ALU.max,
                                )
                            else:
                                nc.scalar.activation(
                                    dst, ps[(half_x, m)], AF.Relu, bias=bias_ap,
                                )
                    if li == 0 and h7_prev is not None:
                        # previous pair's SDF matmuls slot in here: they are
                        # ready to run (h7 relus done) and the L0 block
                        # shadows the relu->matmul dependency.
                        psf_prev = emit_sdf(h7_prev)
                        h7_prev = None
                    if li == 1 and psf_prev is not None:
                        # drain early: the single SDF bank is reused by the
                        # ring at L2's last allocation, and the drain must
                        # precede L2's relus in the DVE FIFO (deadlock
                        # otherwise).
                        emit_sdf_out(p - 1, psf_prev)
                        psf_prev = None
                    if li == 2 and p + 1 < PAIRS:
                        # next pair's embedding DMAs (sync queue)
                        emb_next = emit_embedding(p + 1)
                    if li == 3:
                        h3 = h
                    h_prev = h

                h7_prev = h_prev
                embd = emb_next

            sp = emit_sdf(h7_prev)
            emit_sdf_out(PAIRS - 1, sp)
    nc.compile()
    return nc


def bass_ts(i, size):
    return slice(i * size, (i + 1) * size)


def _prep_maps(points, ws, bs, wsdf, bsdf):
    pts = np.ascontiguousarray(points, dtype=np.float32).reshape(N, 3)
    freqs = (2.0 ** np.arange(NHARM)).astype(np.float32)
    fcol18 = (np.repeat(freqs[None, :], 3, axis=0).reshape(18, 1) / TWO_PI).astype(
        np.float32
    )

    bmat = np.zeros((128, 16), dtype=np.float32)
    for i in range(8):
        for m in range(2):
            bmat[:, i * 2 + m] = bs[i][m * 128:(m + 1) * 128]

    wpack = np.zeros((128, WCOLS), dtype=np.float16)
    w0h = ws[0].astype(np.float16)
    wpack[0:E, OFF_W0:OFF_W0 + 256] = w0h
    wpack[64:64 + E, OFF_W0:OFF_W0 + 256] = w0h
    for li in _K_LAYERS:
        wpack[:, OFF_WK[li]:OFF_WK[li] + 256] = ws[li][0:128, :].astype(np.float16)
        wpack[:, OFF_WK[li] + 256:OFF_WK[li] + 512] = ws[li][128:256, :].astype(
            np.float16
        )
    w4eh = ws[4][0:E, :].astype(np.float16)
    wpack[0:E, OFF_W4E:OFF_W4E + 256] = w4eh
    wpack[64:64 + E, OFF_W4E:OFF_W4E + 256] = w4eh
    wpack[:, OFF_W4A:OFF_W4A + 256] = ws[4][E:E + 128, :].astype(np.float16)
    wpack[:, OFF_W4B:OFF_W4B + 256] = ws[4][E + 128:E + 256, :].astype(np.float16)
    wpack[:, OFF_SDF:OFF_SDF + 1] = wsdf[0:128, :].astype(np.float16)
    wpack[:, OFF_SDF + 1:OFF_SDF + 2] = wsdf[128:256, :].astype(np.float16)

    common = {
        "wpack": wpack,
        "bmat": bmat,
        "bsdf1": np.full((128, 1), float(np.ravel(bsdf)[0]), dtype=np.float32),
    }

    in_maps = []
    for c in range(N_CORES):
        sl = pts[c * NPC:(c + 1) * NPC]  # [NPC, 3]
        ptsT = np.ascontiguousarray(sl.T)  # [3, NPC]
        rep3 = np.repeat(ptsT, NHARM, axis=0)  # [18, NPC]
        t18 = rep3 * fcol18  # x * 2^j / (2pi), exact fp32 scaling
        t36 = np.empty((36, NPC), dtype=np.float32)
        t36[0:18], t36[18:36] = t18, t18 + np.float32(0.25)
        # host-side range reduction + sin: rows 0:18 = sin(x*2^j), rows
        # 18:36 = cos(x*2^j) (phase shift), rows 36:39 = xyz. fp16, same
        # HBM bytes as shipping the phases.
        embF = np.empty((E, NPC), dtype=np.float16)
        embF[0:36] = np.sin(TWO_PI * (t36 - np.round(t36)))
        embF[36:39] = ptsT
        m = dict(common)
        m["embf"] = embF
        in_maps.append(m)
    return in_maps


def kernel(
    points, w0, b0, w1, b1, w2, b2, w3, b3, w4, b4, w5, b5, w6, b6, w7, b7,
    wsdf, bsdf,
):
    ws = [np.asarray(w, dtype=np.float32) for w in (w0, w1, w2, w3, w4, w5, w6, w7)]
    bs = [np.asarray(b, dtype=np.float32) for b in (b0, b1, b2, b3, b4, b5, b6, b7)]
    in_maps = _prep_maps(
        np.asarray(points), ws, bs,
        np.asarray(wsdf, dtype=np.float32), np.asarray(bsdf, dtype=np.float32),
    )

    if "nc" not in _CACHED:
        _CACHED["nc"] = _build()
    nc = _CACHED["nc"]

    res = run_bass_kernel_spmd(nc, in_maps, core_ids=list(range(N_CORES)))
    out = np.concatenate(
        [res.results[c]["out_o"] for c in range(N_CORES)], axis=0
    ).reshape(N, 1).astype(np.float32)
    return out
